# revision 4
# baseline (speedup 1.0000x reference)
"""Trainium2 Bass kernel for nn_MetaSim_56925496541280.

Data-parallel over batch B=64 across 8 NeuronCores (8 batches/core).
Per batch (N=1024 nodes, F=512 feats):
    enc = A @ relu(A @ X @ We1 + be1) @ We2 + be2        [N, 18]
    dec = A @ relu(A @ enc @ Wd1 + bd1) @ Wd2 + bd2      [N, F]
    dec_act = [softmax(dec[:, :32]), sigmoid(dec[:, 32:])]
    sampled = dec_act + m * (sigma * noise)
    log_probs = sum(m * (-0.5*noise^2 - log(sigma) - 0.5*log(2pi)))

Layout trick: every product with A contracts over A's columns, so the host
ships adj pre-transposed (adjT[j, i] = adj[i, j]).  The chain then alternates
node-major / feat-major intermediates so that no on-device transpose is ever
needed:
    t1T  [F, N]   = X.T @ A.T      (lhsT = X node-major,  rhs = A.T)
    h1   [N, 30]  = relu(t1 @ We1) (lhsT = t1T slices,    rhs = We1)
    t2T  [30, N]  = h1.T @ A.T     (lhsT = h1,            rhs = A.T)
    enc  [N, 18]  = t2 @ We2       (lhsT = t2T slices,    rhs = We2)
    t3T  [18, N]  = enc.T @ A.T
    h2   [N, 30]  = relu(t3 @ Wd1)
    t4T  [30, N]  = h2.T @ A.T
    dec  [N, F]   = t4 @ Wd2
Matmuls run as float32r (1 col/cycle vs 4 for fp32; fp32 PSUM accumulate).
"""

import os
import sys

for _p in ("/opt/trn_rl_repo", "/root/.axon_site/_ro/trn_rl_repo"):
    if os.path.isdir(_p) and _p not in sys.path:
        sys.path.insert(0, _p)

import numpy as np
from contextlib import ExitStack

import concourse.bacc as bacc
import concourse.bass as bass
import concourse.tile as tile
from concourse import mybir
from concourse.bass_utils import run_bass_kernel_spmd

F32 = mybir.dt.float32
F32R = mybir.dt.float32r
I32 = mybir.dt.int32
BF16 = mybir.dt.bfloat16
U8 = mybir.dt.uint8

N_CORES = 8
B_FULL = 64
BB = B_FULL // N_CORES  # batches per core
N = 1024
F = 512
H1 = 30   # We1 out
H2 = 18   # We2 out (enc width)
NC = 32   # NUM_CLASSES
SIGMA = 0.02
LOG_SIGMA = float(np.log(SIGMA))
HALF_LOG_2PI = float(0.5 * np.log(2.0 * np.pi))
LP_C = -LOG_SIGMA - HALF_LOG_2PI  # lp_elem = -0.5*n^2 + LP_C

# --- tuning flags ---
A_BF16 = False   # ship adjT / x as bf16 (halves the dominant DMA)
MASK_U8 = False  # ship masks as uint8
MM_DT = F32R     # dtype of the matmul path (F32R = tf32-like, 4x fp32 speed)

_cache = {}


def _mm_dt(ap):
    # float32 tiles are fed to the PE as float32r (same bits, 4x faster)
    return ap.bitcast(F32R) if ap.dtype == F32 else ap


def _build():
    nc = bacc.Bacc("TRN2", target_bir_lowering=False, debug=False)

    in_dt = BF16 if A_BF16 else MM_DT
    m_dt = U8 if MASK_U8 else I32

    adjT_d = nc.declare_dram_parameter("adjT", [BB, N, N], in_dt, isOutput=False)
    x_d = nc.declare_dram_parameter("x", [BB, N, F], in_dt, isOutput=False)
    masks_d = nc.declare_dram_parameter("masks", [BB, N, F], m_dt, isOutput=False)
    noise_d = nc.declare_dram_parameter("noise", [BB, N, F], F32, isOutput=False)
    We1_d = nc.declare_dram_parameter("We1", [F, H1], MM_DT, isOutput=False)
    be1_d = nc.declare_dram_parameter("be1", [H1], F32, isOutput=False)
    We2_d = nc.declare_dram_parameter("We2", [H1, H2], MM_DT, isOutput=False)
    be2_d = nc.declare_dram_parameter("be2", [H2], F32, isOutput=False)
    Wd1_d = nc.declare_dram_parameter("Wd1", [H2, H1], MM_DT, isOutput=False)
    bd1_d = nc.declare_dram_parameter("bd1", [H1], F32, isOutput=False)
    Wd2_d = nc.declare_dram_parameter("Wd2", [H1, F], MM_DT, isOutput=False)
    bd2_d = nc.declare_dram_parameter("bd2", [F], F32, isOutput=False)
    dec_d = nc.declare_dram_parameter("dec", [BB, N, F], F32, isOutput=True)
    samp_d = nc.declare_dram_parameter("sampled", [BB, N, F], F32, isOutput=True)
    lp_d = nc.declare_dram_parameter("lp", [BB], F32, isOutput=True)

    with tile.TileContext(nc) as tc, ExitStack() as ctx:
        _emit(ctx, tc, nc, adjT_d, x_d, masks_d, noise_d,
              We1_d, be1_d, We2_d, be2_d, Wd1_d, bd1_d, Wd2_d, bd2_d,
              dec_d, samp_d, lp_d, in_dt, m_dt)

    nc.compile()
    return nc


def _emit(ctx, tc, nc, adjT_d, x_d, masks_d, noise_d,
          We1_d, be1_d, We2_d, be2_d, Wd1_d, bd1_d, Wd2_d, bd2_d,
          dec_d, samp_d, lp_d, in_dt, m_dt):
    AF = mybir.ActivationFunctionType
    P = 128
    NJ = N // P        # 8 node chunks
    NF = F // P        # 4 feat chunks
    NIH = N // 512     # 2 moving-dim halves

    # ---------------- pools ----------------
    wpool = ctx.enter_context(tc.tile_pool(name="weights", bufs=1))
    apool = ctx.enter_context(tc.tile_pool(name="adjT", bufs=2))
    xpool = ctx.enter_context(tc.tile_pool(name="x", bufs=2))
    mpool = ctx.enter_context(tc.tile_pool(name="masks", bufs=1))
    npool = ctx.enter_context(tc.tile_pool(name="noise", bufs=1))
    t1pool = ctx.enter_context(tc.tile_pool(name="t1T", bufs=1))
    smallpool = ctx.enter_context(tc.tile_pool(name="smalls", bufs=1))
    blkpool = ctx.enter_context(tc.tile_pool(name="blk", bufs=2))
    lppool = ctx.enter_context(tc.tile_pool(name="lp", bufs=1))
    ps_big = ctx.enter_context(tc.tile_pool(name="ps_big", bufs=3, space="PSUM"))
    ps_mid = ctx.enter_context(tc.tile_pool(name="ps_mid", bufs=2, space="PSUM"))
    ps_sml = ctx.enter_context(tc.tile_pool(name="ps_sml", bufs=2, space="PSUM"))
    ps_one = ctx.enter_context(tc.tile_pool(name="ps_one", bufs=1, space="PSUM"))

    # ---------------- weights + constants (once) ----------------
    We1_sb = wpool.tile([P, NF, H1], MM_DT, tag="We1")
    nc.sync.dma_start(We1_sb[:], We1_d.ap().rearrange("(c p) n -> p c n", p=P))
    We2_sb = wpool.tile([H1, H2], MM_DT, tag="We2")
    nc.sync.dma_start(We2_sb[:], We2_d.ap())
    Wd1_sb = wpool.tile([H2, H1], MM_DT, tag="Wd1")
    nc.sync.dma_start(Wd1_sb[:], Wd1_d.ap())
    Wd2_sb = wpool.tile([H1, F], MM_DT, tag="Wd2")
    nc.sync.dma_start(Wd2_sb[:], Wd2_d.ap())

    be1_row = wpool.tile([1, H1], F32, tag="be1r")
    nc.sync.dma_start(be1_row[:], be1_d.ap().unsqueeze(0))
    be2_row = wpool.tile([1, H2], F32, tag="be2r")
    nc.sync.dma_start(be2_row[:], be2_d.ap().unsqueeze(0))
    bd1_row = wpool.tile([1, H1], F32, tag="bd1r")
    nc.sync.dma_start(bd1_row[:], bd1_d.ap().unsqueeze(0))
    bd2_row = wpool.tile([1, F], F32, tag="bd2r")
    nc.sync.dma_start(bd2_row[:], bd2_d.ap().unsqueeze(0))

    ones_row = wpool.tile([1, P], F32, tag="ones_row")
    nc.vector.memset(ones_row[:], 1.0)
    ones_col = wpool.tile([P, 1], F32, tag="ones_col")
    nc.vector.memset(ones_col[:], 1.0)

    # broadcast biases across partitions: ones[1,P].T @ b[1,n] -> [P, n]
    def bias_bcast(row, n, tag):
        ps = ps_one.tile([P, n], F32, tag="ps_one")
        nc.tensor.matmul(ps[:], ones_row[:], row[:], start=True, stop=True)
        sb = wpool.tile([P, n], F32, tag=tag)
        nc.scalar.copy(sb[:], ps[:])
        return sb

    be1_b = bias_bcast(be1_row, H1, "be1b")
    be2_b = bias_bcast(be2_row, H2, "be2b")
    bd1_b = bias_bcast(bd1_row, H1, "bd1b")
    bd2_b = bias_bcast(bd2_row, F, "bd2b")

    lp_acc = lppool.tile([P, BB], F32, tag="lp_acc")
    nc.vector.memset(lp_acc[:], 0.0)

    # ---------------- per-batch pipeline ----------------
    for b in range(BB):
        AT = apool.tile([P, NJ, N], in_dt, tag="AT")
        nc.sync.dma_start(AT[:], adjT_d.ap()[b].rearrange("(c p) i -> p c i", p=P))
        X = xpool.tile([P, NJ, F], in_dt, tag="X")
        nc.sync.dma_start(X[:], x_d.ap()[b].rearrange("(c p) f -> p c f", p=P))
        M = mpool.tile([P, NJ, F], m_dt, tag="M")
        nc.sync.dma_start(M[:], masks_d.ap()[b].rearrange("(c p) f -> p c f", p=P))
        NO = npool.tile([P, NJ, F], F32, tag="NO")
        nc.sync.dma_start(NO[:], noise_d.ap()[b].rearrange("(c p) f -> p c f", p=P))

        # M1: t1T[f, i] = sum_j X[j, f] * A[i, j]
        t1T = t1pool.tile([P, NF, N], MM_DT, tag="t1T")
        for ih in range(NIH):
            for f in range(NF):
                ps = ps_big.tile([P, 512], F32, tag="ps_big")
                for j in range(NJ):
                    nc.tensor.matmul(
                        ps[:],
                        _mm_dt(X[:, j, f * P:(f + 1) * P]),
                        _mm_dt(AT[:, j, ih * 512:(ih + 1) * 512]),
                        start=(j == 0), stop=(j == NJ - 1))
                nc.scalar.copy(t1T[:, f, ih * 512:(ih + 1) * 512], ps[:])

        # M2: h1[i, :] = relu(t1 @ We1 + be1)   (node-major)
        h1 = smallpool.tile([P, NJ, H1], MM_DT, tag="h1")
        for i in range(NJ):
            ps = ps_sml.tile([P, H1], F32, tag="ps_sml")
            for c in range(NF):
                nc.tensor.matmul(
                    ps[:],
                    _mm_dt(t1T[:, c, i * P:(i + 1) * P]),
                    _mm_dt(We1_sb[:, c, :]),
                    start=(c == 0), stop=(c == NF - 1))
            tmp = smallpool.tile([P, H1], F32, tag="h1tmp")
            nc.vector.tensor_add(tmp[:], ps[:], be1_b[:])
            nc.scalar.activation(h1[:, i, :], tmp[:], AF.Relu)

        # M3: t2T[30, i] = sum_j h1[j, :] * A[i, j]
        t2T = smallpool.tile([H1, N], MM_DT, tag="t2T")
        for ih in range(NIH):
            ps = ps_mid.tile([H1, 512], F32, tag="ps_mid")
            for j in range(NJ):
                nc.tensor.matmul(
                    ps[:], _mm_dt(h1[:, j, :]),
                    _mm_dt(AT[:, j, ih * 512:(ih + 1) * 512]),
                    start=(j == 0), stop=(j == NJ - 1))
            nc.scalar.copy(t2T[:, ih * 512:(ih + 1) * 512], ps[:])

        # M4: enc[i, :] = t2 @ We2 + be2   (node-major)
        enc = smallpool.tile([P, NJ, H2], MM_DT, tag="enc")
        for i in range(NJ):
            ps = ps_sml.tile([P, H2], F32, tag="ps_sml")
            nc.tensor.matmul(ps[:], _mm_dt(t2T[:, i * P:(i + 1) * P]),
                             _mm_dt(We2_sb[:]), start=True, stop=True)
            nc.vector.tensor_add(enc[:, i, :], ps[:], be2_b[:])

        # M5: t3T[18, i] = sum_j enc[j, :] * A[i, j]
        t3T = smallpool.tile([H2, N], MM_DT, tag="t3T")
        for ih in range(NIH):
            ps = ps_mid.tile([H2, 512], F32, tag="ps_mid")
            for j in range(NJ):
                nc.tensor.matmul(
                    ps[:], _mm_dt(enc[:, j, :]),
                    _mm_dt(AT[:, j, ih * 512:(ih + 1) * 512]),
                    start=(j == 0), stop=(j == NJ - 1))
            nc.scalar.copy(t3T[:, ih * 512:(ih + 1) * 512], ps[:])

        # M6: h2[i, :] = relu(t3 @ Wd1 + bd1)
        h2 = smallpool.tile([P, NJ, H1], MM_DT, tag="h2")
        for i in range(NJ):
            ps = ps_sml.tile([P, H1], F32, tag="ps_sml")
            nc.tensor.matmul(ps[:], _mm_dt(t3T[:, i * P:(i + 1) * P]),
                             _mm_dt(Wd1_sb[:]), start=True, stop=True)
            tmp = smallpool.tile([P, H1], F32, tag="h2tmp")
            nc.vector.tensor_add(tmp[:], ps[:], bd1_b[:])
            nc.scalar.activation(h2[:, i, :], tmp[:], AF.Relu)

        # M7: t4T[30, i] = sum_j h2[j, :] * A[i, j]
        t4T = smallpool.tile([H1, N], MM_DT, tag="t4T")
        for ih in range(NIH):
            ps = ps_mid.tile([H1, 512], F32, tag="ps_mid")
            for j in range(NJ):
                nc.tensor.matmul(
                    ps[:], _mm_dt(h2[:, j, :]),
                    _mm_dt(AT[:, j, ih * 512:(ih + 1) * 512]),
                    start=(j == 0), stop=(j == NJ - 1))
            nc.scalar.copy(t4T[:, ih * 512:(ih + 1) * 512], ps[:])

        # M8 + sampling tail, per node block
        for i in range(NJ):
            ps = ps_big.tile([P, F], F32, tag="ps_big")
            nc.tensor.matmul(ps[:], _mm_dt(t4T[:, i * P:(i + 1) * P]),
                             _mm_dt(Wd2_sb[:]), start=True, stop=True)
            dec_sb = blkpool.tile([P, F], F32, tag="dec_sb")
            nc.vector.tensor_add(dec_sb[:], ps[:], bd2_b[:])
            nc.sync.dma_start(dec_d.ap()[b, i * P:(i + 1) * P, :], dec_sb[:])

            # dec_act = [softmax(dec[:, :32]), sigmoid(dec[:, 32:])]
            dec_act = blkpool.tile([P, F], F32, tag="dec_act")
            esum = blkpool.tile([P, 1], F32, tag="esum")
            nc.scalar.activation(dec_act[:, :NC], dec_sb[:, :NC], AF.Exp,
                                 accum_out=esum[:])
            erec = blkpool.tile([P, 1], F32, tag="erec")
            nc.vector.reciprocal(erec[:], esum[:])
            nc.vector.tensor_scalar_mul(dec_act[:, :NC], dec_act[:, :NC], erec[:])
            nc.scalar.activation(dec_act[:, NC:], dec_sb[:, NC:], AF.Sigmoid)

            # masks/noise tail
            mf = blkpool.tile([P, F], F32, tag="mf")
            msum = blkpool.tile([P, 1], F32, tag="msum")
            nc.scalar.activation(mf[:], M[:, i, :], AF.Copy, accum_out=msum[:])
            mn = blkpool.tile([P, F], F32, tag="mn")
            nc.vector.tensor_mul(mn[:], mf[:], NO[:, i, :])

            samp = blkpool.tile([P, F], F32, tag="samp")
            nc.vector.scalar_tensor_tensor(
                samp[:], mn[:], SIGMA, dec_act[:],
                op0=mybir.AluOpType.mult, op1=mybir.AluOpType.add)
            nc.sync.dma_start(samp_d.ap()[b, i * P:(i + 1) * P, :], samp[:])

            # lp partial: sum_f(-0.5 * m * n^2) + LP_C * sum_f(m)
            qacc = blkpool.tile([P, 1], F32, tag="qacc")
            nc.vector.scalar_tensor_tensor(
                mf[:], mn[:], -0.5, NO[:, i, :],
                op0=mybir.AluOpType.mult, op1=mybir.AluOpType.mult,
                accum_out=qacc[:])
            contrib = blkpool.tile([P, 1], F32, tag="contrib")
            nc.vector.scalar_tensor_tensor(
                contrib[:], msum[:], LP_C, qacc[:],
                op0=mybir.AluOpType.mult, op1=mybir.AluOpType.add)
            nc.vector.tensor_add(lp_acc[:, b:b + 1], lp_acc[:, b:b + 1], contrib[:])

    # final cross-partition reduce for log_probs: ones[P,1].T @ lp_acc[P,BB]
    ps = ps_one.tile([1, BB], F32, tag="ps_one")
    nc.tensor.matmul(ps[:], ones_col[:], lp_acc[:], start=True, stop=True)
    lp_sb = lppool.tile([1, BB], F32, tag="lp_sb")
    nc.scalar.copy(lp_sb[:], ps[:])
    nc.sync.dma_start(lp_d.ap().unsqueeze(0), lp_sb[:])


def _get_compiled():
    key = (A_BF16, MASK_U8)
    if key not in _cache:
        _cache[key] = _build()
    return _cache[key]


def _make_in_maps(x, adj, masks, noise, We1, be1, We2, be2, Wd1, bd1, Wd2, bd2):
    in_dt = np.dtype("bfloat16") if A_BF16 else np.float32
    if A_BF16:
        import ml_dtypes
        in_dt = ml_dtypes.bfloat16
    adjT = np.ascontiguousarray(adj.transpose(0, 2, 1))
    if A_BF16:
        adjT = adjT.astype(in_dt)
        x = x.astype(in_dt)
    m = masks.astype(np.uint8) if MASK_U8 else np.ascontiguousarray(masks)
    ws = dict(We1=np.ascontiguousarray(We1), be1=np.ascontiguousarray(be1),
              We2=np.ascontiguousarray(We2), be2=np.ascontiguousarray(be2),
              Wd1=np.ascontiguousarray(Wd1), bd1=np.ascontiguousarray(bd1),
              Wd2=np.ascontiguousarray(Wd2), bd2=np.ascontiguousarray(bd2))
    in_maps = []
    for c in range(N_CORES):
        s = slice(c * BB, (c + 1) * BB)
        in_maps.append(dict(adjT=adjT[s], x=np.ascontiguousarray(x[s]),
                            masks=m[s], noise=np.ascontiguousarray(noise[s]),
                            **ws))
    return in_maps


def kernel(x, adj, masks, noise, We1, be1, We2, be2, Wd1, bd1, Wd2, bd2):
    x = np.asarray(x, dtype=np.float32)
    adj = np.asarray(adj, dtype=np.float32)
    masks = np.asarray(masks)
    noise = np.asarray(noise, dtype=np.float32)
    nc = _get_compiled()
    in_maps = _make_in_maps(x, adj, masks, noise, We1, be1, We2, be2,
                            Wd1, bd1, Wd2, bd2)
    res = run_bass_kernel_spmd(nc, in_maps, core_ids=list(range(N_CORES)))
    dec = np.concatenate([r["dec"] for r in res.results], axis=0)
    sampled = np.concatenate([r["sampled"] for r in res.results], axis=0)
    log_probs = np.concatenate([r["lp"] for r in res.results], axis=0)
    return dec, sampled, log_probs


if __name__ == "__main__":
    _get_compiled()
    print("build OK")
